# revision 20
# baseline (speedup 1.0000x reference)
"""DCNv3 forward on 8 trn2 NeuronCores.

Strategy (data-parallel over (batch, H-half) -> 8 shards):
  - host: pre-transpose per-shard input into the layouts the device wants
    (zero-padded fp16 pixel slab with (cc,g)-interleaved channels for
    sampling; CHW fp16 tile for the offset/mask matmuls). The (g,p) axis is
    padded 36->40 (pitch 10) so every fp16 DVE op is 4B-aligned (2x_1p mode).
  - device per core, per RT-row tile:
      PE matmul  : offsets + mask logits for RT*128 pixels (fp16, 120 cols)
      ACT        : exp, hat weights relu(1 - |u - t|) for a COMMON absolute
                   tap window (UxV cells shared by every (g,p))
      DVE (fp16) : per-cell coefficients A[q,g,u,v] = sum_p m_p*hy_p(u)*hx_p(v)
                   via one big outer-product mult + one reduce over p (pitch
                   10 keeps both in 2x mode); apply = per used cell one
                   RT*128-elem mult (2x_1p: all inner dims stride-1 thanks to
                   the channel interleave), then a binary add tree over cells.
  - bilinear gather is gather-free: integer parts of all sampling offsets are
    bounded, so sampling = hat-weighted fixed shifts of the input; every fixed
    shift is an access-pattern offset into an SBUF slab (V column-shifted
    copies of the row slab).
"""

import numpy as np
import sys

sys.path.insert(0, "/opt/trn_rl_repo")

import concourse.bass as bass
import concourse.bacc as bacc
import concourse.mybir as mybir
import concourse.tile as tile
from concourse.bass_utils import run_bass_kernel_spmd

B, C, H, W = 4, 128, 128, 128
G, P, gc = 4, 9, 32
N_CORES = 8
HS = H // 2          # rows per core (b, half)
RT = 8               # steady-state output rows per device tile
# small leading/trailing tiles shorten the pipeline fill and drain
TILE_RTS = [2, 2, 4] + [8] * ((HS - 16) // RT) + [4, 2, 2]
assert sum(TILE_RTS) == HS

GP = 40              # padded (g,p) pitch: col = g*10 + p, p=9 is a zero pad
RQ = 3 * GP          # rawq pitch: y | x | logits sections

f32 = mybir.dt.float32
f16 = mybir.dt.float16

_KS = np.array([-1.0, 0.0, 1.0], np.float32)
KX = np.repeat(_KS, 3)   # x-major flatten (matches torch meshgrid in ref)
KY = np.tile(_KS, 3)


def _geometry(inp, W_off, b_off):
    """Global tap window + used-cell mask from the actual offset field."""
    xhw = inp.reshape(B, H, W, C)
    off = (xhw.reshape(-1, C) @ W_off + b_off).reshape(-1, G, P, 2)
    rx = off[..., 0] + KX          # offset (x) relative to wo+1  (padded coords)
    ry = off[..., 1] + KY
    Bx = np.floor(rx.min(axis=0)).astype(np.int64)
    By = np.floor(ry.min(axis=0)).astype(np.int64)
    spx = np.floor(rx.max(axis=0)).astype(np.int64) + 2 - Bx
    spy = np.floor(ry.max(axis=0)).astype(np.int64) + 2 - By
    g = _Geom()
    g.DX0 = int(Bx.min())
    g.DY0 = int(By.min())
    g.V = int((Bx + spx).max()) - g.DX0
    g.U = int((By + spy).max()) - g.DY0
    used = np.zeros((g.U, g.V), bool)
    for gg in range(G):
        for p in range(P):
            u0 = By[gg, p] - g.DY0
            v0 = Bx[gg, p] - g.DX0
            used[u0:u0 + spy[gg, p], v0:v0 + spx[gg, p]] = True
    g.cells = [(u, v) for u in range(g.U) for v in range(g.V) if used[u, v]]
    return g


class _Geom:
    pass


def _build(g: "_Geom"):
    nc = bacc.Bacc("TRN2", target_bir_lowering=False, debug=False,
                   num_devices=N_CORES)

    U, V = g.U, g.V
    NCELL = len(g.cells)
    NTAP = max(U, V)

    xslab_t = nc.dram_tensor("xslab", [g.NROW * g.NCOL * C], f16, kind="ExternalInput")
    xchw_t = nc.dram_tensor("xchw", [C, HS * W], f16, kind="ExternalInput")
    wcat_t = nc.dram_tensor("wcat", [C, RQ], f16, kind="ExternalInput")
    addc_t = nc.dram_tensor("addc", [C, RQ], f32, kind="ExternalInput")
    cvals_t = nc.dram_tensor("cvals", [C, NTAP + 1], f32, kind="ExternalInput")
    out_t = nc.dram_tensor("out", [HS * W * C], f16, kind="ExternalOutput")

    mult, add = mybir.AluOpType.mult, mybir.AluOpType.add
    AF = mybir.ActivationFunctionType

    def vap(v, off, dims):
        return bass.AP(tensor=v.tensor, offset=v.offset + off, ap=[v.ap[0]] + dims)

    with tile.TileContext(nc) as tc:
        with (
            nc.allow_low_precision(reason="fp16 cell sums, fp32 positions"),
            tc.tile_pool(name="const", bufs=1) as cpool,
            tc.tile_pool(name="xs", bufs=2) as xspool,
            tc.tile_pool(name="work", bufs=2) as wpool,
            tc.tile_pool(name="big", bufs=1) as bpool,
            tc.tile_pool(name="psum", bufs=4, space="PSUM") as pspool,
        ):
            wcat0 = cpool.tile([C, RQ], f16)
            wcat = cpool.tile([C, RQ], f16)
            addc = cpool.tile([C, RQ], f32)
            cvals = cpool.tile([C, NTAP + 1], f32)
            nc.sync.dma_start(wcat0[:], wcat_t.ap())
            nc.sync.dma_start(addc[:], addc_t.ap())
            nc.sync.dma_start(cvals[:], cvals_t.ap())
            # matmul operands come via ACT copies: the Matmult HW struct has a
            # single sync-wait slot, so all its deps must arrive on one sem
            nc.scalar.copy(wcat[:], wcat0[:])

            row0 = 0
            for rt in TILE_RTS:
                nr = rt + 1 + g.DY0 + U   # slab rows this tile (rho max + 1)
                rgp = rt * GP
                rg = rt * G

                # ---- loads -------------------------------------------------
                xs = xspool.tile([C, V * nr * C], f16, name="xs")
                for v in range(V):
                    src = bass.AP(
                        tensor=xslab_t,
                        offset=(row0 * g.NCOL + g.C0 + v) * C,
                        ap=[[C, W], [g.NCOL * C, nr], [1, C]])
                    nc.sync.dma_start(
                        vap(xs[:], v * nr * C, [[C, nr], [1, C]]), src)

                xc0 = wpool.tile([C, rt * W], f16, name="xc0")
                nc.sync.dma_start(
                    xc0[:], bass.AP(tensor=xchw_t, offset=row0 * W,
                                    ap=[[HS * W, C], [1, rt * W]]))
                xc = wpool.tile([C, rt * W], f16, name="xc")
                nc.scalar.copy(xc[:], xc0[:])

                # ---- offsets / logits (PE), directly in q-on-partitions ----
                rawq = wpool.tile([C, rt * RQ], f32, name="rawq")
                for k in range(rt):
                    praw = pspool.tile([C, RQ], f32, name="praw")
                    nc.tensor.matmul(praw[:], xc[:, k * W:(k + 1) * W],
                                     wcat[:], start=True, stop=True)
                    nc.scalar.copy(vap(rawq[:], k * RQ, [[1, RQ]]), praw[:])

                # rawq[:, r*RQ + k] : k 0..39 y-offs, 40..79 x-offs, 80..119 logits
                nc.vector.tensor_tensor(
                    vap(rawq[:], 0, [[RQ, rt], [1, RQ]]),
                    vap(rawq[:], 0, [[RQ, rt], [1, RQ]]),
                    vap(addc[:], 0, [[0, rt], [1, RQ]]), add)

                # ---- softmax (unnormalized exp -> normalize) ---------------
                el = wpool.tile([C, rt * GP], f16, name="el")
                nc.scalar.activation(
                    vap(el[:], 0, [[GP, rt], [1, GP]]),
                    vap(rawq[:], 2 * GP, [[RQ, rt], [1, GP]]), AF.Exp)
                den = wpool.tile([C, rt * G], f32, name="den")
                nc.vector.tensor_reduce(
                    vap(den[:], 0, [[G, rt], [1, G]]),
                    vap(el[:], 0, [[GP, rt], [10, G], [1, P]]),
                    mybir.AxisListType.X, add)
                denr = wpool.tile([C, rt * G], f32, name="denr")
                nc.vector.reciprocal(denr[:], den[:])
                nc.vector.tensor_tensor(
                    vap(el[:], 0, [[GP, rt], [10, G], [1, P]]),
                    vap(el[:], 0, [[GP, rt], [10, G], [1, P]]),
                    vap(denr[:], 0, [[G, rt], [1, G], [0, P]]), mult)

                # ---- hat weights at absolute taps 0..NTAP-1 (y and x) ------
                # hyx[wo, t*rt*80 + r*80 + (0..39 y | 40..79 x)]
                hyx = bpool.tile([C, NTAP * rt * 2 * GP], f16, name="hyx")
                habs = bpool.tile([C, NTAP * rt * 2 * GP], f16, name="habs")
                for tt in range(NTAP):
                    nc.scalar.activation(
                        vap(habs[:], tt * rt * 2 * GP, [[2 * GP, rt], [1, 2 * GP]]),
                        vap(rawq[:], 0, [[RQ, rt], [1, 2 * GP]]),
                        AF.Abs, bias=cvals[:, tt:tt + 1])
                    nc.scalar.activation(
                        vap(hyx[:], tt * rt * 2 * GP, [[2 * GP, rt], [1, 2 * GP]]),
                        vap(habs[:], tt * rt * 2 * GP, [[2 * GP, rt], [1, 2 * GP]]),
                        AF.Relu, bias=cvals[:, NTAP:NTAP + 1], scale=-1.0)

                # compact hx: hxc[wo, v*rt*GP + r*GP + gp]
                hxc = wpool.tile([C, V * rgp], f16, name="hxc")
                nc.scalar.copy(
                    vap(hxc[:], 0, [[rgp, V], [GP, rt], [1, GP]]),
                    vap(hyx[:], GP, [[rt * 2 * GP, V], [2 * GP, rt], [1, GP]]))

                # ---- mh[u] = m * hy_u  (one op over (u, r, gp)) ------------
                mh = wpool.tile([C, U * rgp], f16, name="mh")
                nc.vector.tensor_tensor(
                    vap(mh[:], 0, [[rgp, U], [GP, rt], [1, GP]]),
                    vap(el[:], 0, [[0, U], [GP, rt], [1, GP]]),
                    vap(hyx[:], 0, [[rt * 2 * GP, U], [2 * GP, rt], [1, GP]]), mult)

                # ---- A[q,g,u,v] = sum_p mh_u,p * hx_v,p --------------------
                # prod[wo, (u*V+v)*rgp + r*GP + gp]
                prod = bpool.tile([C, U * V * rgp], f16, name="prod")
                nc.vector.tensor_tensor(
                    vap(prod[:], 0, [[V * rgp, U], [rgp, V], [1, rgp]]),
                    vap(mh[:], 0, [[rgp, U], [0, V], [1, rgp]]),
                    vap(hxc[:], 0, [[0, U], [rgp, V], [1, rgp]]), mult)
                # A[wo, (u*V+v)*rg + r*4 + g] = sum_p prod.  tensor_reduce is
                # 1x-only on DVE; a short TT tree stays in 2x_1p instead.
                # q[wo, (uv, rg, 0:4)] = prod[p 0:4] + prod[p 4:8]
                q = wpool.tile([C, U * V * rg * 4], f16, name="q")
                nc.vector.tensor_tensor(
                    vap(q[:], 0, [[rg * 4, U * V], [4, rg], [1, 4]]),
                    vap(prod[:], 0, [[rgp, U * V], [10, rg], [1, 4]]),
                    vap(prod[:], 4, [[rgp, U * V], [10, rg], [1, 4]]), add)
                # q[.., 0:2] += prod[p 8:10]  (p=9 pad contributes ~0)
                nc.vector.tensor_tensor(
                    vap(q[:], 0, [[rg * 4, U * V], [4, rg], [1, 2]]),
                    vap(q[:], 0, [[rg * 4, U * V], [4, rg], [1, 2]]),
                    vap(prod[:], 8, [[rgp, U * V], [10, rg], [1, 2]]), add)
                # q[.., 0:2] += q[.., 2:4]
                nc.vector.tensor_tensor(
                    vap(q[:], 0, [[rg * 4, U * V], [4, rg], [1, 2]]),
                    vap(q[:], 0, [[rg * 4, U * V], [4, rg], [1, 2]]),
                    vap(q[:], 2, [[rg * 4, U * V], [4, rg], [1, 2]]), add)
                # A = q[.., 0] + q[.., 1]
                A = wpool.tile([C, U * V * rg], f16, name="A")
                nc.vector.tensor_tensor(
                    vap(A[:], 0, [[rg, U * V], [1, rg]]),
                    vap(q[:], 0, [[rg * 4, U * V], [4, rg]]),
                    vap(q[:], 1, [[rg * 4, U * V], [4, rg]]), add)

                # ---- apply -------------------------------------------------
                # channels are host-interleaved: slab channel index = cc*G + gg
                # tmp[wo, j*rt*C + r*128 + cc*4 + g]  (contiguous per cell)
                tmp = bpool.tile([C, NCELL * rt * C], f16, name="tmp")
                for j, (u, v) in enumerate(g.cells):
                    rho0 = 2 + g.DY0 + u
                    nc.vector.tensor_tensor(
                        vap(tmp[:], j * rt * C,
                            [[C, rt], [G, gc], [1, G]]),
                        vap(xs[:], (v * nr + rho0) * C,
                            [[C, rt], [G, gc], [1, G]]),
                        vap(A[:], (u * V + v) * rg,
                            [[G, rt], [0, gc], [1, G]]), mult)

                # binary tree of contiguous adds over cells -> tmp[:, 0:rt*C]
                n = NCELL
                while n > 1:
                    half = n // 2
                    nc.vector.tensor_tensor(
                        vap(tmp[:], 0, [[1, half * rt * C]]),
                        vap(tmp[:], 0, [[1, half * rt * C]]),
                        vap(tmp[:], half * rt * C, [[1, half * rt * C]]), add)
                    n = half

                nc.sync.dma_start(
                    bass.AP(tensor=out_t, offset=row0 * W * C,
                            ap=[[C, W], [W * C, rt], [1, C]]),
                    vap(tmp[:], 0, [[C, rt], [1, C]]))

                row0 += rt

    nc.compile()
    return nc


def _host_prep(inp, W_off, b_off, W_mask, b_mask, g):
    xhw = inp.reshape(B, H, W, C)
    NTAP = max(g.U, g.V)

    wcat = np.zeros((C, RQ), np.float32)
    addc_row = np.zeros(RQ, np.float32)
    addc_row[2 * GP:] = -30.0        # pad logits -> exp ~ 0
    for gg in range(G):
        for p in range(P):
            gp = gg * 10 + p
            wcat[:, gp] = W_off[:, 2 * (gg * P + p) + 1]           # y
            wcat[:, GP + gp] = W_off[:, 2 * (gg * P + p)]          # x
            wcat[:, 2 * GP + gp] = W_mask[:, gg * P + p]
            addc_row[gp] = b_off[2 * (gg * P + p) + 1] + (KY[p] - g.DY0)
            addc_row[GP + gp] = b_off[2 * (gg * P + p)] + (KX[p] - g.DX0)
            addc_row[2 * GP + gp] = b_mask[gg * P + p]
    addc = np.tile(addc_row[None, :], (C, 1))
    cvals = np.zeros((C, NTAP + 1), np.float32)
    for i in range(NTAP):
        cvals[:, i] = -float(i)
    cvals[:, NTAP] = 1.0

    # channel interleave: new channel index cc*G + gg  <- old gg*gc + cc
    perm = np.arange(C).reshape(G, gc).T.reshape(-1)   # perm[new] = old

    in_maps = []
    for core in range(N_CORES):
        b, half = divmod(core, 2)
        h0 = HS * half
        # slab rows: padded rows [h0-1, h0-1+NROW) ; cols: padded [-2, NCOL-2)
        xslab = np.zeros((g.NROW, g.NCOL, C), np.float16)
        for lr in range(g.NROW):
            orig = lr + h0 - 2
            if 0 <= orig < H:
                xslab[lr, 3:3 + W, :] = xhw[b, orig][:, perm].astype(np.float16)
        xchw = np.ascontiguousarray(
            xhw[b, h0:h0 + HS].reshape(HS * W, C).T).astype(np.float16)
        in_maps.append({
            "xslab": xslab.reshape(-1),
            "xchw": xchw,
            "wcat": wcat.astype(np.float16),
            "addc": addc,
            "cvals": cvals,
        })
    return in_maps


def _make_geom(inp, W_off, b_off):
    g = _geometry(inp, W_off, b_off)
    # slab row for output row r (in tile), tap u: rho = r + 2 + DY0 + u
    rmin = 2 + g.DY0
    assert rmin >= 0
    # slab rows per core: last tile's row0 + its row span
    g.NROW = (HS - TILE_RTS[-1]) + TILE_RTS[-1] + 1 + g.DY0 + g.U
    # slab col for (wo, v): wo + v + (3 + DX0)
    g.C0 = 3 + g.DX0                      # col offset baked into slab layout
    assert g.C0 >= 0
    g.NCOL = W + g.V - 1 + g.C0 + 1
    return g


def _run(inp, W_off, b_off, W_mask, b_mask, **spmd_kwargs):
    inp = np.ascontiguousarray(inp, np.float32)
    g = _make_geom(inp, np.asarray(W_off, np.float32), np.asarray(b_off, np.float32))
    nc = _build(g)
    in_maps = _host_prep(inp, np.asarray(W_off, np.float32),
                         np.asarray(b_off, np.float32),
                         np.asarray(W_mask, np.float32),
                         np.asarray(b_mask, np.float32), g)
    res = run_bass_kernel_spmd(nc, in_maps, core_ids=list(range(N_CORES)),
                               **spmd_kwargs)
    # inverse channel interleave: out channel position cc*G + gg
    perm = np.arange(C).reshape(G, gc).T.reshape(-1)
    inv = np.empty(C, np.int64)
    inv[perm] = np.arange(C)
    out = np.empty((B, H, W, C), np.float32)
    for core in range(N_CORES):
        b, half = divmod(core, 2)
        o = res.results[core]["out"].astype(np.float32).reshape(HS, W, C)
        out[b, HS * half:HS * (half + 1)] = o[:, :, inv]
    return out.reshape(B, C, H, W), res


def kernel(inp, W_off, b_off, W_mask, b_mask):
    out, _ = _run(inp, W_off, b_off, W_mask, b_mask)
    return out


if __name__ == "__main__":
    d = np.load("/root/problem/ref_cache.npz")
    got = kernel(d["inp"], d["W_off"], d["b_off"], d["W_mask"], d["b_mask"])
    exp = d["exp"]
    err = np.abs(got - exp).max()
    print("absmax err:", err, "rel:", err / np.abs(exp).max())


# revision 21
# speedup vs baseline: 1.0152x; 1.0152x over previous
"""DCNv3 forward on 8 trn2 NeuronCores.

Strategy (data-parallel over (batch, H-half) -> 8 shards):
  - host: pre-transpose per-shard input into the layouts the device wants
    (zero-padded fp16 pixel slab with (cc,g)-interleaved channels for
    sampling; CHW fp16 tile for the offset/mask matmuls). The (g,p) axis is
    padded 36->40 (pitch 10) so every fp16 DVE op is 4B-aligned (2x_1p mode).
  - device per core, per RT-row tile:
      PE matmul  : offsets + mask logits for RT*128 pixels (fp16, 120 cols)
      ACT        : exp, hat weights relu(1 - |u - t|) for a COMMON absolute
                   tap window (UxV cells shared by every (g,p))
      DVE (fp16) : per-cell coefficients A[q,g,u,v] = sum_p m_p*hy_p(u)*hx_p(v)
                   via one big outer-product mult + one reduce over p (pitch
                   10 keeps both in 2x mode); apply = per used cell one
                   RT*128-elem mult (2x_1p: all inner dims stride-1 thanks to
                   the channel interleave), then a binary add tree over cells.
  - bilinear gather is gather-free: integer parts of all sampling offsets are
    bounded, so sampling = hat-weighted fixed shifts of the input; every fixed
    shift is an access-pattern offset into an SBUF slab (V column-shifted
    copies of the row slab).
"""

import numpy as np
import sys

sys.path.insert(0, "/opt/trn_rl_repo")

import concourse.bass as bass
import concourse.bacc as bacc
import concourse.mybir as mybir
import concourse.tile as tile
from concourse.bass_utils import run_bass_kernel_spmd

B, C, H, W = 4, 128, 128, 128
G, P, gc = 4, 9, 32
N_CORES = 8
HS = H // 2          # rows per core (b, half)
RT = 8               # steady-state output rows per device tile
# small leading tiles shorten the pipeline-fill serial chain
TILE_RTS = [2, 2, 4] + [8] * ((HS - 8) // RT)
assert sum(TILE_RTS) == HS

GP = 40              # padded (g,p) pitch: col = g*10 + p, p=9 is a zero pad
RQ = 3 * GP          # rawq pitch: y | x | logits sections

f32 = mybir.dt.float32
f16 = mybir.dt.float16

_KS = np.array([-1.0, 0.0, 1.0], np.float32)
KX = np.repeat(_KS, 3)   # x-major flatten (matches torch meshgrid in ref)
KY = np.tile(_KS, 3)


def _geometry(inp, W_off, b_off):
    """Global tap window + used-cell mask from the actual offset field."""
    xhw = inp.reshape(B, H, W, C)
    off = (xhw.reshape(-1, C) @ W_off + b_off).reshape(-1, G, P, 2)
    rx = off[..., 0] + KX          # offset (x) relative to wo+1  (padded coords)
    ry = off[..., 1] + KY
    Bx = np.floor(rx.min(axis=0)).astype(np.int64)
    By = np.floor(ry.min(axis=0)).astype(np.int64)
    spx = np.floor(rx.max(axis=0)).astype(np.int64) + 2 - Bx
    spy = np.floor(ry.max(axis=0)).astype(np.int64) + 2 - By
    g = _Geom()
    g.DX0 = int(Bx.min())
    g.DY0 = int(By.min())
    g.V = int((Bx + spx).max()) - g.DX0
    g.U = int((By + spy).max()) - g.DY0
    used = np.zeros((g.U, g.V), bool)
    for gg in range(G):
        for p in range(P):
            u0 = By[gg, p] - g.DY0
            v0 = Bx[gg, p] - g.DX0
            used[u0:u0 + spy[gg, p], v0:v0 + spx[gg, p]] = True
    g.cells = [(u, v) for u in range(g.U) for v in range(g.V) if used[u, v]]
    return g


class _Geom:
    pass


def _build(g: "_Geom"):
    nc = bacc.Bacc("TRN2", target_bir_lowering=False, debug=False,
                   num_devices=N_CORES)

    U, V = g.U, g.V
    NCELL = len(g.cells)
    NTAP = max(U, V)

    xslab_t = nc.dram_tensor("xslab", [g.NROW * g.NCOL * C], f16, kind="ExternalInput")
    xchw_t = nc.dram_tensor("xchw", [C, HS * W], f16, kind="ExternalInput")
    wcat_t = nc.dram_tensor("wcat", [C, RQ], f16, kind="ExternalInput")
    addc_t = nc.dram_tensor("addc", [C, RQ], f32, kind="ExternalInput")
    cvals_t = nc.dram_tensor("cvals", [C, NTAP + 1], f32, kind="ExternalInput")
    out_t = nc.dram_tensor("out", [HS * W * C], f16, kind="ExternalOutput")

    mult, add = mybir.AluOpType.mult, mybir.AluOpType.add
    AF = mybir.ActivationFunctionType

    def vap(v, off, dims):
        return bass.AP(tensor=v.tensor, offset=v.offset + off, ap=[v.ap[0]] + dims)

    with tile.TileContext(nc) as tc:
        with (
            nc.allow_low_precision(reason="fp16 cell sums, fp32 positions"),
            tc.tile_pool(name="const", bufs=1) as cpool,
            tc.tile_pool(name="xs", bufs=2) as xspool,
            tc.tile_pool(name="work", bufs=2) as wpool,
            tc.tile_pool(name="big", bufs=1) as bpool,
            tc.tile_pool(name="psum", bufs=4, space="PSUM") as pspool,
        ):
            wcat0 = cpool.tile([C, RQ], f16)
            wcat = cpool.tile([C, RQ], f16)
            addc = cpool.tile([C, RQ], f32)
            cvals = cpool.tile([C, NTAP + 1], f32)
            nc.sync.dma_start(wcat0[:], wcat_t.ap())
            nc.sync.dma_start(addc[:], addc_t.ap())
            nc.sync.dma_start(cvals[:], cvals_t.ap())
            # matmul operands come via ACT copies: the Matmult HW struct has a
            # single sync-wait slot, so all its deps must arrive on one sem
            nc.scalar.copy(wcat[:], wcat0[:])

            row0 = 0
            for rt in TILE_RTS:
                nr = rt + 1 + g.DY0 + U   # slab rows this tile (rho max + 1)
                rgp = rt * GP
                rg = rt * G

                # ---- loads -------------------------------------------------
                xs = xspool.tile([C, V * nr * C], f16, name="xs")
                for v in range(V):
                    src = bass.AP(
                        tensor=xslab_t,
                        offset=(row0 * g.NCOL + g.C0 + v) * C,
                        ap=[[C, W], [g.NCOL * C, nr], [1, C]])
                    nc.sync.dma_start(
                        vap(xs[:], v * nr * C, [[C, nr], [1, C]]), src)

                xc0 = wpool.tile([C, rt * W], f16, name="xc0")
                nc.sync.dma_start(
                    xc0[:], bass.AP(tensor=xchw_t, offset=row0 * W,
                                    ap=[[HS * W, C], [1, rt * W]]))
                xc = wpool.tile([C, rt * W], f16, name="xc")
                nc.scalar.copy(xc[:], xc0[:])

                # ---- offsets / logits (PE), directly in q-on-partitions ----
                rawq = wpool.tile([C, rt * RQ], f32, name="rawq")
                for k in range(rt):
                    praw = pspool.tile([C, RQ], f32, name="praw")
                    nc.tensor.matmul(praw[:], xc[:, k * W:(k + 1) * W],
                                     wcat[:], start=True, stop=True)
                    nc.scalar.copy(vap(rawq[:], k * RQ, [[1, RQ]]), praw[:])

                # rawq[:, r*RQ + k] : k 0..39 y-offs, 40..79 x-offs, 80..119 logits
                nc.vector.tensor_tensor(
                    vap(rawq[:], 0, [[RQ, rt], [1, RQ]]),
                    vap(rawq[:], 0, [[RQ, rt], [1, RQ]]),
                    vap(addc[:], 0, [[0, rt], [1, RQ]]), add)

                # ---- softmax (unnormalized exp -> normalize) ---------------
                el = wpool.tile([C, rt * GP], f16, name="el")
                nc.scalar.activation(
                    vap(el[:], 0, [[GP, rt], [1, GP]]),
                    vap(rawq[:], 2 * GP, [[RQ, rt], [1, GP]]), AF.Exp)
                den = wpool.tile([C, rt * G], f32, name="den")
                nc.vector.tensor_reduce(
                    vap(den[:], 0, [[G, rt], [1, G]]),
                    vap(el[:], 0, [[GP, rt], [10, G], [1, P]]),
                    mybir.AxisListType.X, add)
                denr = wpool.tile([C, rt * G], f32, name="denr")
                nc.vector.reciprocal(denr[:], den[:])
                nc.vector.tensor_tensor(
                    vap(el[:], 0, [[GP, rt], [10, G], [1, P]]),
                    vap(el[:], 0, [[GP, rt], [10, G], [1, P]]),
                    vap(denr[:], 0, [[G, rt], [1, G], [0, P]]), mult)

                # ---- hat weights at absolute taps 0..NTAP-1 (y and x) ------
                # hyx[wo, t*rt*80 + r*80 + (0..39 y | 40..79 x)]
                hyx = bpool.tile([C, NTAP * rt * 2 * GP], f16, name="hyx")
                habs = bpool.tile([C, NTAP * rt * 2 * GP], f16, name="habs")
                for tt in range(NTAP):
                    nc.scalar.activation(
                        vap(habs[:], tt * rt * 2 * GP, [[2 * GP, rt], [1, 2 * GP]]),
                        vap(rawq[:], 0, [[RQ, rt], [1, 2 * GP]]),
                        AF.Abs, bias=cvals[:, tt:tt + 1])
                    nc.scalar.activation(
                        vap(hyx[:], tt * rt * 2 * GP, [[2 * GP, rt], [1, 2 * GP]]),
                        vap(habs[:], tt * rt * 2 * GP, [[2 * GP, rt], [1, 2 * GP]]),
                        AF.Relu, bias=cvals[:, NTAP:NTAP + 1], scale=-1.0)

                # compact hx: hxc[wo, v*rt*GP + r*GP + gp]
                hxc = wpool.tile([C, V * rgp], f16, name="hxc")
                nc.scalar.copy(
                    vap(hxc[:], 0, [[rgp, V], [GP, rt], [1, GP]]),
                    vap(hyx[:], GP, [[rt * 2 * GP, V], [2 * GP, rt], [1, GP]]))

                # ---- mh[u] = m * hy_u  (one op over (u, r, gp)) ------------
                mh = wpool.tile([C, U * rgp], f16, name="mh")
                nc.vector.tensor_tensor(
                    vap(mh[:], 0, [[rgp, U], [GP, rt], [1, GP]]),
                    vap(el[:], 0, [[0, U], [GP, rt], [1, GP]]),
                    vap(hyx[:], 0, [[rt * 2 * GP, U], [2 * GP, rt], [1, GP]]), mult)

                # ---- A[q,g,u,v] = sum_p mh_u,p * hx_v,p --------------------
                # prod[wo, (u*V+v)*rgp + r*GP + gp]
                prod = bpool.tile([C, U * V * rgp], f16, name="prod")
                nc.vector.tensor_tensor(
                    vap(prod[:], 0, [[V * rgp, U], [rgp, V], [1, rgp]]),
                    vap(mh[:], 0, [[rgp, U], [0, V], [1, rgp]]),
                    vap(hxc[:], 0, [[0, U], [rgp, V], [1, rgp]]), mult)
                # A[wo, (u*V+v)*rg + r*4 + g] = sum_p prod.  tensor_reduce is
                # 1x-only on DVE; a short TT tree stays in 2x_1p instead.
                # q[wo, (uv, rg, 0:4)] = prod[p 0:4] + prod[p 4:8]
                q = wpool.tile([C, U * V * rg * 4], f16, name="q")
                nc.vector.tensor_tensor(
                    vap(q[:], 0, [[rg * 4, U * V], [4, rg], [1, 4]]),
                    vap(prod[:], 0, [[rgp, U * V], [10, rg], [1, 4]]),
                    vap(prod[:], 4, [[rgp, U * V], [10, rg], [1, 4]]), add)
                # q[.., 0:2] += prod[p 8:10]  (p=9 pad contributes ~0)
                nc.vector.tensor_tensor(
                    vap(q[:], 0, [[rg * 4, U * V], [4, rg], [1, 2]]),
                    vap(q[:], 0, [[rg * 4, U * V], [4, rg], [1, 2]]),
                    vap(prod[:], 8, [[rgp, U * V], [10, rg], [1, 2]]), add)
                # q[.., 0:2] += q[.., 2:4]
                nc.vector.tensor_tensor(
                    vap(q[:], 0, [[rg * 4, U * V], [4, rg], [1, 2]]),
                    vap(q[:], 0, [[rg * 4, U * V], [4, rg], [1, 2]]),
                    vap(q[:], 2, [[rg * 4, U * V], [4, rg], [1, 2]]), add)
                # A = q[.., 0] + q[.., 1]
                A = wpool.tile([C, U * V * rg], f16, name="A")
                nc.vector.tensor_tensor(
                    vap(A[:], 0, [[rg, U * V], [1, rg]]),
                    vap(q[:], 0, [[rg * 4, U * V], [4, rg]]),
                    vap(q[:], 1, [[rg * 4, U * V], [4, rg]]), add)

                # ---- apply -------------------------------------------------
                # channels are host-interleaved: slab channel index = cc*G + gg
                # tmp[wo, j*rt*C + r*128 + cc*4 + g]  (contiguous per cell)
                tmp = bpool.tile([C, NCELL * rt * C], f16, name="tmp")
                for j, (u, v) in enumerate(g.cells):
                    rho0 = 2 + g.DY0 + u
                    nc.vector.tensor_tensor(
                        vap(tmp[:], j * rt * C,
                            [[C, rt], [G, gc], [1, G]]),
                        vap(xs[:], (v * nr + rho0) * C,
                            [[C, rt], [G, gc], [1, G]]),
                        vap(A[:], (u * V + v) * rg,
                            [[G, rt], [0, gc], [1, G]]), mult)

                # binary tree of contiguous adds over cells -> tmp[:, 0:rt*C]
                n = NCELL
                while n > 1:
                    half = n // 2
                    nc.vector.tensor_tensor(
                        vap(tmp[:], 0, [[1, half * rt * C]]),
                        vap(tmp[:], 0, [[1, half * rt * C]]),
                        vap(tmp[:], half * rt * C, [[1, half * rt * C]]), add)
                    n = half

                nc.sync.dma_start(
                    bass.AP(tensor=out_t, offset=row0 * W * C,
                            ap=[[C, W], [W * C, rt], [1, C]]),
                    vap(tmp[:], 0, [[C, rt], [1, C]]))

                row0 += rt

    nc.compile()
    return nc


def _host_prep(inp, W_off, b_off, W_mask, b_mask, g):
    xhw = inp.reshape(B, H, W, C)
    NTAP = max(g.U, g.V)

    wcat = np.zeros((C, RQ), np.float32)
    addc_row = np.zeros(RQ, np.float32)
    addc_row[2 * GP:] = -30.0        # pad logits -> exp ~ 0
    for gg in range(G):
        for p in range(P):
            gp = gg * 10 + p
            wcat[:, gp] = W_off[:, 2 * (gg * P + p) + 1]           # y
            wcat[:, GP + gp] = W_off[:, 2 * (gg * P + p)]          # x
            wcat[:, 2 * GP + gp] = W_mask[:, gg * P + p]
            addc_row[gp] = b_off[2 * (gg * P + p) + 1] + (KY[p] - g.DY0)
            addc_row[GP + gp] = b_off[2 * (gg * P + p)] + (KX[p] - g.DX0)
            addc_row[2 * GP + gp] = b_mask[gg * P + p]
    addc = np.tile(addc_row[None, :], (C, 1))
    cvals = np.zeros((C, NTAP + 1), np.float32)
    for i in range(NTAP):
        cvals[:, i] = -float(i)
    cvals[:, NTAP] = 1.0

    # channel interleave: new channel index cc*G + gg  <- old gg*gc + cc
    perm = np.arange(C).reshape(G, gc).T.reshape(-1)   # perm[new] = old

    in_maps = []
    for core in range(N_CORES):
        b, half = divmod(core, 2)
        h0 = HS * half
        # slab rows: padded rows [h0-1, h0-1+NROW) ; cols: padded [-2, NCOL-2)
        xslab = np.zeros((g.NROW, g.NCOL, C), np.float16)
        for lr in range(g.NROW):
            orig = lr + h0 - 2
            if 0 <= orig < H:
                xslab[lr, 3:3 + W, :] = xhw[b, orig][:, perm].astype(np.float16)
        xchw = np.ascontiguousarray(
            xhw[b, h0:h0 + HS].reshape(HS * W, C).T).astype(np.float16)
        in_maps.append({
            "xslab": xslab.reshape(-1),
            "xchw": xchw,
            "wcat": wcat.astype(np.float16),
            "addc": addc,
            "cvals": cvals,
        })
    return in_maps


def _make_geom(inp, W_off, b_off):
    g = _geometry(inp, W_off, b_off)
    # slab row for output row r (in tile), tap u: rho = r + 2 + DY0 + u
    rmin = 2 + g.DY0
    assert rmin >= 0
    # slab rows per core: last tile's row0 + its row span
    g.NROW = (HS - TILE_RTS[-1]) + TILE_RTS[-1] + 1 + g.DY0 + g.U
    # slab col for (wo, v): wo + v + (3 + DX0)
    g.C0 = 3 + g.DX0                      # col offset baked into slab layout
    assert g.C0 >= 0
    g.NCOL = W + g.V - 1 + g.C0 + 1
    return g


def _run(inp, W_off, b_off, W_mask, b_mask, **spmd_kwargs):
    inp = np.ascontiguousarray(inp, np.float32)
    g = _make_geom(inp, np.asarray(W_off, np.float32), np.asarray(b_off, np.float32))
    nc = _build(g)
    in_maps = _host_prep(inp, np.asarray(W_off, np.float32),
                         np.asarray(b_off, np.float32),
                         np.asarray(W_mask, np.float32),
                         np.asarray(b_mask, np.float32), g)
    res = run_bass_kernel_spmd(nc, in_maps, core_ids=list(range(N_CORES)),
                               **spmd_kwargs)
    # inverse channel interleave: out channel position cc*G + gg
    perm = np.arange(C).reshape(G, gc).T.reshape(-1)
    inv = np.empty(C, np.int64)
    inv[perm] = np.arange(C)
    out = np.empty((B, H, W, C), np.float32)
    for core in range(N_CORES):
        b, half = divmod(core, 2)
        o = res.results[core]["out"].astype(np.float32).reshape(HS, W, C)
        out[b, HS * half:HS * (half + 1)] = o[:, :, inv]
    return out.reshape(B, C, H, W), res


def kernel(inp, W_off, b_off, W_mask, b_mask):
    out, _ = _run(inp, W_off, b_off, W_mask, b_mask)
    return out


if __name__ == "__main__":
    d = np.load("/root/problem/ref_cache.npz")
    got = kernel(d["inp"], d["W_off"], d["b_off"], d["W_mask"], d["b_mask"])
    exp = d["exp"]
    err = np.abs(got - exp).max()
    print("absmax err:", err, "rel:", err / np.abs(exp).max())


# revision 23
# speedup vs baseline: 1.0152x; 1.0000x over previous
"""DCNv3 forward on 8 trn2 NeuronCores.

Strategy (data-parallel over (batch, H-half) -> 8 shards):
  - host: pre-transpose per-shard input into the layouts the device wants
    (zero-padded fp16 pixel slab with (cc,g)-interleaved channels for
    sampling; CHW fp16 tile for the offset/mask matmuls). The (g,p) axis is
    padded 36->40 (pitch 10) so every fp16 DVE op is 4B-aligned (2x_1p mode).
  - device per core, per RT-row tile:
      PE matmul  : offsets + mask logits for RT*128 pixels (fp16, 120 cols)
      ACT        : exp, hat weights relu(1 - |u - t|) for a COMMON absolute
                   tap window (UxV cells shared by every (g,p))
      DVE (fp16) : per-cell coefficients A[q,g,u,v] = sum_p m_p*hy_p(u)*hx_p(v)
                   via one big outer-product mult + one reduce over p (pitch
                   10 keeps both in 2x mode); apply = per used cell one
                   RT*128-elem mult (2x_1p: all inner dims stride-1 thanks to
                   the channel interleave), then a binary add tree over cells.
  - bilinear gather is gather-free: integer parts of all sampling offsets are
    bounded, so sampling = hat-weighted fixed shifts of the input; every fixed
    shift is an access-pattern offset into an SBUF slab (V column-shifted
    copies of the row slab).
"""

import numpy as np
import sys

sys.path.insert(0, "/opt/trn_rl_repo")

import concourse.bass as bass
import concourse.bacc as bacc
import concourse.mybir as mybir
import concourse.tile as tile
from concourse.bass_utils import run_bass_kernel_spmd

B, C, H, W = 4, 128, 128, 128
G, P, gc = 4, 9, 32
N_CORES = 8
HS = H // 2          # rows per core (b, half)
RT = 8               # steady-state output rows per device tile
# small leading tiles shorten the pipeline-fill serial chain
TILE_RTS = [2, 2, 4] + [8] * ((HS - 8) // RT)
assert sum(TILE_RTS) == HS

GP = 40              # padded (g,p) pitch: col = g*10 + p, p=9 is a zero pad
RQ = 3 * GP          # rawq pitch: y | x | logits sections

f32 = mybir.dt.float32
f16 = mybir.dt.float16

_KS = np.array([-1.0, 0.0, 1.0], np.float32)
KX = np.repeat(_KS, 3)   # x-major flatten (matches torch meshgrid in ref)
KY = np.tile(_KS, 3)


def _geometry(inp, W_off, b_off):
    """Global tap window + used-cell mask from the actual offset field."""
    xhw = inp.reshape(B, H, W, C)
    off = (xhw.reshape(-1, C) @ W_off + b_off).reshape(-1, G, P, 2)
    rx = off[..., 0] + KX          # offset (x) relative to wo+1  (padded coords)
    ry = off[..., 1] + KY
    Bx = np.floor(rx.min(axis=0)).astype(np.int64)
    By = np.floor(ry.min(axis=0)).astype(np.int64)
    spx = np.floor(rx.max(axis=0)).astype(np.int64) + 2 - Bx
    spy = np.floor(ry.max(axis=0)).astype(np.int64) + 2 - By
    g = _Geom()
    g.DX0 = int(Bx.min())
    g.DY0 = int(By.min())
    g.V = int((Bx + spx).max()) - g.DX0
    g.U = int((By + spy).max()) - g.DY0
    used = np.zeros((g.U, g.V), bool)
    for gg in range(G):
        for p in range(P):
            u0 = By[gg, p] - g.DY0
            v0 = Bx[gg, p] - g.DX0
            used[u0:u0 + spy[gg, p], v0:v0 + spx[gg, p]] = True
    g.cells = [(u, v) for u in range(g.U) for v in range(g.V) if used[u, v]]
    return g


class _Geom:
    pass


def _build(g: "_Geom"):
    nc = bacc.Bacc("TRN2", target_bir_lowering=False, debug=False,
                   num_devices=N_CORES)

    U, V = g.U, g.V
    NCELL = len(g.cells)
    NTAP = max(U, V)

    xslab_t = nc.dram_tensor("xslab", [g.NROW * g.NCOL * C], f16, kind="ExternalInput")
    xchw_t = nc.dram_tensor("xchw", [C, HS * W], f16, kind="ExternalInput")
    wcat_t = nc.dram_tensor("wcat", [C, RQ], f16, kind="ExternalInput")
    addc_t = nc.dram_tensor("addc", [C, RQ], f32, kind="ExternalInput")
    cvals_t = nc.dram_tensor("cvals", [C, NTAP + 1], f32, kind="ExternalInput")
    out_t = nc.dram_tensor("out", [HS * W * C], f16, kind="ExternalOutput")

    mult, add = mybir.AluOpType.mult, mybir.AluOpType.add
    AF = mybir.ActivationFunctionType

    def vap(v, off, dims):
        return bass.AP(tensor=v.tensor, offset=v.offset + off, ap=[v.ap[0]] + dims)

    with tile.TileContext(nc) as tc:
        with (
            nc.allow_low_precision(reason="fp16 cell sums, fp32 positions"),
            tc.tile_pool(name="const", bufs=1) as cpool,
            tc.tile_pool(name="xs", bufs=2) as xspool,
            tc.tile_pool(name="work", bufs=2) as wpool,
            tc.tile_pool(name="big", bufs=1) as bpool,
            tc.tile_pool(name="psum", bufs=4, space="PSUM") as pspool,
        ):
            wcat0 = cpool.tile([C, RQ], f16)
            wcat = cpool.tile([C, RQ], f16)
            addc = cpool.tile([C, RQ], f32)
            cvals = cpool.tile([C, NTAP + 1], f32)
            nc.sync.dma_start(wcat0[:], wcat_t.ap())
            nc.sync.dma_start(addc[:], addc_t.ap())
            nc.sync.dma_start(cvals[:], cvals_t.ap())
            # matmul operands come via ACT copies: the Matmult HW struct has a
            # single sync-wait slot, so all its deps must arrive on one sem
            nc.scalar.copy(wcat[:], wcat0[:])

            row0 = 0
            for rt in TILE_RTS:
                nr = rt + 1 + g.DY0 + U   # slab rows this tile (rho max + 1)
                rgp = rt * GP
                rg = rt * G

                # ---- loads (one tile per column shift: mults for shift v
                # can start as soon as slab v lands) -------------------------
                xsv = []
                for v in range(V):
                    xs1 = xspool.tile([C, nr * C], f16, name=f"xs{v}")
                    src = bass.AP(
                        tensor=xslab_t,
                        offset=(row0 * g.NCOL + g.C0 + v) * C,
                        ap=[[C, W], [g.NCOL * C, nr], [1, C]])
                    nc.sync.dma_start(xs1[:], src)
                    xsv.append(xs1)

                xc0 = wpool.tile([C, rt * W], f16, name="xc0")
                nc.sync.dma_start(
                    xc0[:], bass.AP(tensor=xchw_t, offset=row0 * W,
                                    ap=[[HS * W, C], [1, rt * W]]))
                xc = wpool.tile([C, rt * W], f16, name="xc")
                nc.scalar.copy(xc[:], xc0[:])

                # ---- offsets / logits (PE), directly in q-on-partitions ----
                rawq = wpool.tile([C, rt * RQ], f32, name="rawq")
                for k in range(rt):
                    praw = pspool.tile([C, RQ], f32, name="praw")
                    nc.tensor.matmul(praw[:], xc[:, k * W:(k + 1) * W],
                                     wcat[:], start=True, stop=True)
                    nc.scalar.copy(vap(rawq[:], k * RQ, [[1, RQ]]), praw[:])

                # rawq[:, r*RQ + k] : k 0..39 y-offs, 40..79 x-offs, 80..119 logits
                nc.vector.tensor_tensor(
                    vap(rawq[:], 0, [[RQ, rt], [1, RQ]]),
                    vap(rawq[:], 0, [[RQ, rt], [1, RQ]]),
                    vap(addc[:], 0, [[0, rt], [1, RQ]]), add)

                # ---- softmax (unnormalized exp -> normalize) ---------------
                el = wpool.tile([C, rt * GP], f16, name="el")
                nc.scalar.activation(
                    vap(el[:], 0, [[GP, rt], [1, GP]]),
                    vap(rawq[:], 2 * GP, [[RQ, rt], [1, GP]]), AF.Exp)
                den = wpool.tile([C, rt * G], f32, name="den")
                nc.vector.tensor_reduce(
                    vap(den[:], 0, [[G, rt], [1, G]]),
                    vap(el[:], 0, [[GP, rt], [10, G], [1, P]]),
                    mybir.AxisListType.X, add)
                denr = wpool.tile([C, rt * G], f32, name="denr")
                nc.vector.reciprocal(denr[:], den[:])
                nc.vector.tensor_tensor(
                    vap(el[:], 0, [[GP, rt], [10, G], [1, P]]),
                    vap(el[:], 0, [[GP, rt], [10, G], [1, P]]),
                    vap(denr[:], 0, [[G, rt], [1, G], [0, P]]), mult)

                # ---- hat weights at absolute taps 0..NTAP-1 (y and x) ------
                # hyx[wo, t*rt*80 + r*80 + (0..39 y | 40..79 x)]
                hyx = bpool.tile([C, NTAP * rt * 2 * GP], f16, name="hyx")
                habs = bpool.tile([C, NTAP * rt * 2 * GP], f16, name="habs")
                for tt in range(NTAP):
                    nc.scalar.activation(
                        vap(habs[:], tt * rt * 2 * GP, [[2 * GP, rt], [1, 2 * GP]]),
                        vap(rawq[:], 0, [[RQ, rt], [1, 2 * GP]]),
                        AF.Abs, bias=cvals[:, tt:tt + 1])
                    nc.scalar.activation(
                        vap(hyx[:], tt * rt * 2 * GP, [[2 * GP, rt], [1, 2 * GP]]),
                        vap(habs[:], tt * rt * 2 * GP, [[2 * GP, rt], [1, 2 * GP]]),
                        AF.Relu, bias=cvals[:, NTAP:NTAP + 1], scale=-1.0)

                # compact hx: hxc[wo, v*rt*GP + r*GP + gp]
                hxc = wpool.tile([C, V * rgp], f16, name="hxc")
                nc.scalar.copy(
                    vap(hxc[:], 0, [[rgp, V], [GP, rt], [1, GP]]),
                    vap(hyx[:], GP, [[rt * 2 * GP, V], [2 * GP, rt], [1, GP]]))

                # ---- mh[u] = m * hy_u  (one op over (u, r, gp)) ------------
                mh = wpool.tile([C, U * rgp], f16, name="mh")
                nc.vector.tensor_tensor(
                    vap(mh[:], 0, [[rgp, U], [GP, rt], [1, GP]]),
                    vap(el[:], 0, [[0, U], [GP, rt], [1, GP]]),
                    vap(hyx[:], 0, [[rt * 2 * GP, U], [2 * GP, rt], [1, GP]]), mult)

                # ---- A[q,g,u,v] = sum_p mh_u,p * hx_v,p --------------------
                # prod[wo, (u*V+v)*rgp + r*GP + gp]
                prod = bpool.tile([C, U * V * rgp], f16, name="prod")
                nc.vector.tensor_tensor(
                    vap(prod[:], 0, [[V * rgp, U], [rgp, V], [1, rgp]]),
                    vap(mh[:], 0, [[rgp, U], [0, V], [1, rgp]]),
                    vap(hxc[:], 0, [[0, U], [rgp, V], [1, rgp]]), mult)
                # A[wo, (u*V+v)*rg + r*4 + g] = sum_p prod.  tensor_reduce is
                # 1x-only on DVE; a short TT tree stays in 2x_1p instead.
                # q[wo, (uv, rg, 0:4)] = prod[p 0:4] + prod[p 4:8]
                q = wpool.tile([C, U * V * rg * 4], f16, name="q")
                nc.vector.tensor_tensor(
                    vap(q[:], 0, [[rg * 4, U * V], [4, rg], [1, 4]]),
                    vap(prod[:], 0, [[rgp, U * V], [10, rg], [1, 4]]),
                    vap(prod[:], 4, [[rgp, U * V], [10, rg], [1, 4]]), add)
                # q[.., 0:2] += prod[p 8:10]  (p=9 pad contributes ~0)
                nc.vector.tensor_tensor(
                    vap(q[:], 0, [[rg * 4, U * V], [4, rg], [1, 2]]),
                    vap(q[:], 0, [[rg * 4, U * V], [4, rg], [1, 2]]),
                    vap(prod[:], 8, [[rgp, U * V], [10, rg], [1, 2]]), add)
                # q[.., 0:2] += q[.., 2:4]
                nc.vector.tensor_tensor(
                    vap(q[:], 0, [[rg * 4, U * V], [4, rg], [1, 2]]),
                    vap(q[:], 0, [[rg * 4, U * V], [4, rg], [1, 2]]),
                    vap(q[:], 2, [[rg * 4, U * V], [4, rg], [1, 2]]), add)
                # A = q[.., 0] + q[.., 1]
                A = wpool.tile([C, U * V * rg], f16, name="A")
                nc.vector.tensor_tensor(
                    vap(A[:], 0, [[rg, U * V], [1, rg]]),
                    vap(q[:], 0, [[rg * 4, U * V], [4, rg]]),
                    vap(q[:], 1, [[rg * 4, U * V], [4, rg]]), add)

                # ---- apply -------------------------------------------------
                # channels are host-interleaved: slab channel index = cc*G + gg
                # tmp[wo, j*rt*C + r*128 + cc*4 + g]  (contiguous per cell)
                tmp = bpool.tile([C, NCELL * rt * C], f16, name="tmp")
                for j, (u, v) in enumerate(g.cells):
                    rho0 = 2 + g.DY0 + u
                    nc.vector.tensor_tensor(
                        vap(tmp[:], j * rt * C,
                            [[C, rt], [G, gc], [1, G]]),
                        vap(xsv[v][:], rho0 * C,
                            [[C, rt], [G, gc], [1, G]]),
                        vap(A[:], (u * V + v) * rg,
                            [[G, rt], [0, gc], [1, G]]), mult)

                # binary tree of contiguous adds over cells -> tmp[:, 0:rt*C]
                n = NCELL
                while n > 1:
                    half = n // 2
                    nc.vector.tensor_tensor(
                        vap(tmp[:], 0, [[1, half * rt * C]]),
                        vap(tmp[:], 0, [[1, half * rt * C]]),
                        vap(tmp[:], half * rt * C, [[1, half * rt * C]]), add)
                    n = half

                nc.sync.dma_start(
                    bass.AP(tensor=out_t, offset=row0 * W * C,
                            ap=[[C, W], [W * C, rt], [1, C]]),
                    vap(tmp[:], 0, [[C, rt], [1, C]]))

                row0 += rt

    nc.compile()
    return nc


def _host_prep(inp, W_off, b_off, W_mask, b_mask, g):
    xhw = inp.reshape(B, H, W, C)
    NTAP = max(g.U, g.V)

    wcat = np.zeros((C, RQ), np.float32)
    addc_row = np.zeros(RQ, np.float32)
    addc_row[2 * GP:] = -30.0        # pad logits -> exp ~ 0
    for gg in range(G):
        for p in range(P):
            gp = gg * 10 + p
            wcat[:, gp] = W_off[:, 2 * (gg * P + p) + 1]           # y
            wcat[:, GP + gp] = W_off[:, 2 * (gg * P + p)]          # x
            wcat[:, 2 * GP + gp] = W_mask[:, gg * P + p]
            addc_row[gp] = b_off[2 * (gg * P + p) + 1] + (KY[p] - g.DY0)
            addc_row[GP + gp] = b_off[2 * (gg * P + p)] + (KX[p] - g.DX0)
            addc_row[2 * GP + gp] = b_mask[gg * P + p]
    addc = np.tile(addc_row[None, :], (C, 1))
    cvals = np.zeros((C, NTAP + 1), np.float32)
    for i in range(NTAP):
        cvals[:, i] = -float(i)
    cvals[:, NTAP] = 1.0

    # channel interleave: new channel index cc*G + gg  <- old gg*gc + cc
    perm = np.arange(C).reshape(G, gc).T.reshape(-1)   # perm[new] = old

    in_maps = []
    for core in range(N_CORES):
        b, half = divmod(core, 2)
        h0 = HS * half
        # slab rows: padded rows [h0-1, h0-1+NROW) ; cols: padded [-2, NCOL-2)
        xslab = np.zeros((g.NROW, g.NCOL, C), np.float16)
        for lr in range(g.NROW):
            orig = lr + h0 - 2
            if 0 <= orig < H:
                xslab[lr, 3:3 + W, :] = xhw[b, orig][:, perm].astype(np.float16)
        xchw = np.ascontiguousarray(
            xhw[b, h0:h0 + HS].reshape(HS * W, C).T).astype(np.float16)
        in_maps.append({
            "xslab": xslab.reshape(-1),
            "xchw": xchw,
            "wcat": wcat.astype(np.float16),
            "addc": addc,
            "cvals": cvals,
        })
    return in_maps


def _make_geom(inp, W_off, b_off):
    g = _geometry(inp, W_off, b_off)
    # slab row for output row r (in tile), tap u: rho = r + 2 + DY0 + u
    rmin = 2 + g.DY0
    assert rmin >= 0
    # slab rows per core: last tile's row0 + its row span
    g.NROW = (HS - TILE_RTS[-1]) + TILE_RTS[-1] + 1 + g.DY0 + g.U
    # slab col for (wo, v): wo + v + (3 + DX0)
    g.C0 = 3 + g.DX0                      # col offset baked into slab layout
    assert g.C0 >= 0
    g.NCOL = W + g.V - 1 + g.C0 + 1
    return g


def _run(inp, W_off, b_off, W_mask, b_mask, **spmd_kwargs):
    inp = np.ascontiguousarray(inp, np.float32)
    g = _make_geom(inp, np.asarray(W_off, np.float32), np.asarray(b_off, np.float32))
    nc = _build(g)
    in_maps = _host_prep(inp, np.asarray(W_off, np.float32),
                         np.asarray(b_off, np.float32),
                         np.asarray(W_mask, np.float32),
                         np.asarray(b_mask, np.float32), g)
    res = run_bass_kernel_spmd(nc, in_maps, core_ids=list(range(N_CORES)),
                               **spmd_kwargs)
    # inverse channel interleave: out channel position cc*G + gg
    perm = np.arange(C).reshape(G, gc).T.reshape(-1)
    inv = np.empty(C, np.int64)
    inv[perm] = np.arange(C)
    out = np.empty((B, H, W, C), np.float32)
    for core in range(N_CORES):
        b, half = divmod(core, 2)
        o = res.results[core]["out"].astype(np.float32).reshape(HS, W, C)
        out[b, HS * half:HS * (half + 1)] = o[:, :, inv]
    return out.reshape(B, C, H, W), res


def kernel(inp, W_off, b_off, W_mask, b_mask):
    out, _ = _run(inp, W_off, b_off, W_mask, b_mask)
    return out


if __name__ == "__main__":
    d = np.load("/root/problem/ref_cache.npz")
    got = kernel(d["inp"], d["W_off"], d["b_off"], d["W_mask"], d["b_mask"])
    exp = d["exp"]
    err = np.abs(got - exp).max()
    print("absmax err:", err, "rel:", err / np.abs(exp).max())
